# revision 28
# baseline (speedup 1.0000x reference)
"""Trainium2 Bass kernel for AnchorPlusContrastiveLoss (8 NeuronCores).

v3 design — no collective, everything replicated:
  Anchor: core c handles batch b=c//2, i-rows [h*1024,(h+1)*1024), h=c%2.
    P = 2x_i.x_j - r_i - r_j via one K=6 matmul
    (lhsT=[2x0,2x1,sq0,sq1,-1,-1], rhs=[x0,x1,-1,-1,sq0,sq1]) plus V*m via
    a K=128 fp8 matmul (diag=240, mask {0,1} fp8).  ScalarE
    exp(P/10 - 24) with accum_out yields per-partition sums of e*m
    (masked-out lanes underflow to exactly 0).
  Mask count: the fp8 mask is fetched a second time through four SWDGE
    cast+accumulate DMA chains into bf16 accumulators; a small DVE merge
    tree + one cache-reduce finishes the count.
  CE: every core computes all-batch cluster means locally from the full
    contr_emb (bf16) + host one-hot labels; per-batch strips of one
    [128,64] PSUM tile accumulate rnorm-weighted cluster sums; cluster
    counts via 4 tiny colsum matmuls; rnorm via DVE Newton rsqrt (no
    Ln/Exp table ping-pong); logits for own 1024 rows in [row, b*K]
    orientation; 1/count folded into meansC, rnorm folded into PE diag
    transposes of cT.
  Host combine: sums tiny per-core partials (scalars only).
"""

import numpy as np

import concourse.bacc as bacc
import concourse.bass as bass
import concourse.tile as tile
from concourse import mybir
from concourse.bass_utils import run_bass_kernel_spmd

F32 = mybir.dt.float32
BF16 = mybir.dt.bfloat16
FP8 = mybir.dt.float8e4
U32 = mybir.dt.uint32
I32 = mybir.dt.int32
ALU = mybir.AluOpType
ACT = mybir.ActivationFunctionType

B, N, D, C, K = 4, 2048, 2, 64, 32
NC = 8
ROWS = N // 2          # 1024 anchor i-rows per core
NT = ROWS // 128       # 8 i-tiles per core
NU = (B * N) // 128    # 64 row-chunks of the full contr_emb
NUO = ROWS // 128      # 8 own row-chunks for the CE tail
VMASK = 240.0          # fp8-exact mask amplitude; exp(-24) ~ 3.8e-11
TEMP = 10.0
CE_W = 10.0
MAGIC = 0x5F3759DF

_cached_nc = None


def _expand(ap, reps, where):
    """Insert a 0-stride dim of length `reps` at position `where`."""
    aps = list(ap.ap)
    aps.insert(where, [0, reps])
    return bass.AP(tensor=ap.tensor, offset=ap.offset, ap=aps)


def build():
    nc = bacc.Bacc("TRN2", target_bir_lowering=False, debug=False, num_devices=NC)

    maskf = nc.declare_dram_parameter("maskf", [ROWS, N], FP8, isOutput=False)
    masku = nc.declare_dram_parameter("masku", [ROWS, N], FP8, isOutput=False)
    embjT = nc.declare_dram_parameter("embjT", [D, N], BF16, isOutput=False)
    crdjT = nc.declare_dram_parameter("crdjT", [D, N], BF16, isOutput=False)
    embiT = nc.declare_dram_parameter("embiT", [D, ROWS], BF16, isOutput=False)
    crdiT = nc.declare_dram_parameter("crdiT", [D, ROWS], BF16, isOutput=False)
    cef = nc.declare_dram_parameter("cef", [128, NU * C], BF16, isOutput=False)
    oh = nc.declare_dram_parameter("oh", [128, NU * K], BF16, isOutput=False)
    cefo = nc.declare_dram_parameter("cefo", [128, NUO * C], BF16, isOutput=False)
    oho = nc.declare_dram_parameter("oho", [128, NUO * K], BF16, isOutput=False)
    identf8 = nc.declare_dram_parameter("identf8", [128, 128], FP8, isOutput=False)
    ident1 = nc.declare_dram_parameter("ident1", [128, 128], BF16, isOutput=False)
    onesb = nc.declare_dram_parameter("onesb", [128, 1], BF16, isOutput=False)
    negones = nc.declare_dram_parameter("negones", [2, N], BF16, isOutput=False)
    out_ext = nc.declare_dram_parameter("out", [128, 34], F32, isOutput=True)

    with tile.TileContext(nc) as tc:
        with (
            tc.tile_pool(name="singles", bufs=1) as sg,
            tc.tile_pool(name="work", bufs=1) as wk,
            tc.tile_pool(name="maskp", bufs=3) as mp,
            tc.tile_pool(name="ep", bufs=2) as ep,
            tc.tile_pool(name="psA", bufs=2, space="PSUM") as psA,
            tc.tile_pool(name="psB", bufs=1, space="PSUM") as psB,
            tc.tile_pool(name="psM", bufs=1, space="PSUM") as psM,
            tc.tile_pool(name="psL", bufs=1, space="PSUM") as psL,
        ):
            # ---- ScalarE exp-table preload via dummy act ----
            dume = sg.tile([1, 1], F32)
            nc.vector.memset(dume[:], 1.0)
            dumo = sg.tile([1, 1], F32)
            nc.scalar.activation(dumo[:], dume[:], ACT.Exp)

            # ---- small input DMAs (scalar queue — idle until act0) ----
            lhsT6 = sg.tile([6, ROWS], BF16)
            rhs6 = sg.tile([6, N], BF16)
            etj = wk.tile([2, N], BF16)
            nc.scalar.dma_start(out=etj[:], in_=embjT.ap())
            atj = wk.tile([2, N], BF16)
            nc.scalar.dma_start(out=atj[:], in_=crdjT.ap())
            eti = wk.tile([2, ROWS], BF16)
            nc.scalar.dma_start(out=eti[:], in_=embiT.ap())
            ati = wk.tile([2, ROWS], BF16)
            nc.scalar.dma_start(out=ati[:], in_=crdiT.ap())
            nc.scalar.dma_start(out=lhsT6[4:6, :], in_=negones.ap()[:, 0:ROWS])
            nc.scalar.dma_start(out=rhs6[2:4, :], in_=negones.ap())
            t_if8 = sg.tile([128, 128], FP8)
            nc.scalar.dma_start(out=t_if8[:], in_=identf8.ap())
            t_id1 = sg.tile([128, 128], BF16)
            nc.scalar.dma_start(out=t_id1[:], in_=ident1.ap())
            t_ones = sg.tile([128, 1], BF16)
            nc.scalar.dma_start(out=t_ones[:], in_=onesb.ap())

            # ---------------- anchor operand prep (all DVE + 2 DMAs) ----
            xt = wk.tile([2, N], BF16)
            nc.vector.tensor_tensor(xt[:], etj[:], atj[:], ALU.add)
            nc.vector.tensor_copy(rhs6[0:2, :], xt[:])
            sqj = wk.tile([2, N], BF16)
            nc.vector.tensor_tensor(sqj[:], xt[:], xt[:], ALU.mult)
            nc.scalar.dma_start(out=rhs6[4:6, :], in_=sqj[:])

            xo = wk.tile([2, ROWS], BF16)
            nc.vector.tensor_tensor(xo[:], eti[:], ati[:], ALU.add)
            nc.vector.tensor_scalar(lhsT6[0:2, :], xo[:], 2.0, None, ALU.mult)
            sqi = wk.tile([2, ROWS], BF16)
            nc.vector.tensor_tensor(sqi[:], xo[:], xo[:], ALU.mult)
            nc.scalar.dma_start(out=lhsT6[2:4, :], in_=sqi[:])

            # ---------------- bulk input DMAs (sync queue) -------------
            mt = []
            for t in range(NT):
                m = mp.tile([128, N], FP8, tag="mask")
                nc.sync.dma_start(
                    out=m[:], in_=maskf.ap().rearrange("(t p) n -> t p n", p=128)[t]
                )
                mt.append(m)
                if t == 0:
                    ceft = sg.tile([128, NU, C], BF16)
                    nc.sync.dma_start(
                        out=ceft[:],
                        in_=cef.ap().rearrange("p (u c) -> p u c", u=NU),
                    )
                if t == 1:
                    oht = sg.tile([128, NU, K], BF16)
                    nc.sync.dma_start(
                        out=oht[:], in_=oh.ap().rearrange("p (u k) -> p u k", u=NU)
                    )
                    cefot = sg.tile([128, NUO, C], BF16)
                    nc.sync.dma_start(
                        out=cefot[:],
                        in_=cefo.ap().rearrange("p (u c) -> p u c", u=NUO),
                    )
                    ohot = sg.tile([128, NUO, K], BF16)
                    nc.sync.dma_start(
                        out=ohot[:],
                        in_=oho.ap().rearrange("p (u k) -> p u k", u=NUO),
                    )

            # ---- mask count: four cast+accumulate SWDGE chains --------
            mc = [
                sg.tile([128, N], BF16, name=f"mc{i}", tag=f"mc{i}")
                for i in range(4)
            ]
            mview = masku.ap().rearrange("(t p) n -> t p n", p=128)
            for t in range(NT):
                nc.gpsimd.dma_start(
                    out=mc[t % 4][:],
                    in_=mview[t],
                    accum_op=(ALU.bypass if t < 4 else ALU.add),
                )

            # ---------------- anchor main loop --------------------------
            emsum = sg.tile([128, 2 * NT], F32)
            bias24 = sg.tile([128, 1], F32)
            nc.vector.memset(bias24[:], -VMASK / TEMP)
            for t in range(NT):
                for h in range(2):
                    pt = psA.tile([128, 1024], F32, tag="anchor")
                    for q in range(2):
                        cs = slice((2 * h + q) * 512, (2 * h + q + 1) * 512)
                        nc.tensor.matmul(
                            pt[:, q * 512 : (q + 1) * 512],
                            lhsT6[:, t * 128 : (t + 1) * 128],
                            rhs6[:, cs],
                            start=True, stop=False,
                            skip_group_check=True,
                        )
                        nc.tensor.matmul(
                            pt[:, q * 512 : (q + 1) * 512],
                            t_if8[:],
                            mt[t][:, cs],
                            start=False, stop=True,
                            skip_group_check=True,
                        )
                    je = ep.tile([128, 1024], BF16, tag="junk")
                    nc.scalar.activation(
                        je[:], pt[:], ACT.Exp,
                        bias=bias24[:],
                        scale=1.0 / TEMP,
                        accum_out=emsum[:, 2 * t + h : 2 * t + h + 1],
                    )

            # ---------------- CE head -----------------------------------
            # squared norms for all B*N rows: bf16 square + tree reduce
            sq = sg.tile([128, NU, C], BF16)
            nc.vector.tensor_tensor(sq[:], ceft[:], ceft[:], ALU.mult)
            s32 = sg.tile([128, NU, 32], BF16)
            nc.vector.tensor_tensor(
                s32[:],
                sq[:].rearrange("p u (a c) -> p u a c", a=2)[:, :, 0],
                sq[:].rearrange("p u (a c) -> p u a c", a=2)[:, :, 1],
                ALU.add,
            )
            s16 = sg.tile([128, NU, 16], BF16)
            nc.vector.tensor_tensor(
                s16[:],
                s32[:].rearrange("p u (a c) -> p u a c", a=2)[:, :, 0],
                s32[:].rearrange("p u (a c) -> p u a c", a=2)[:, :, 1],
                ALU.add,
            )
            n2 = sg.tile([128, NU], F32)
            nc.vector.tensor_reduce(n2[:], s16[:], mybir.AxisListType.X, ALU.add)
            # own-row norms (separate small input)
            sqo = sg.tile([128, NUO, C], BF16)
            nc.vector.tensor_tensor(sqo[:], cefot[:], cefot[:], ALU.mult)
            so32 = sg.tile([128, NUO, 32], BF16)
            nc.vector.tensor_tensor(
                so32[:],
                sqo[:].rearrange("p u (a c) -> p u a c", a=2)[:, :, 0],
                sqo[:].rearrange("p u (a c) -> p u a c", a=2)[:, :, 1],
                ALU.add,
            )
            n2o = sg.tile([128, NUO], F32)
            nc.vector.tensor_reduce(n2o[:], so32[:], mybir.AxisListType.X, ALU.add)

            # rnorm = rsqrt(n2) via magic-constant Newton (2 iterations)
            def rsqrt(dst, src, cols):
                half = wk.tile([128, cols], F32, tag=f"nh{cols}")
                nc.vector.tensor_scalar(half[:], src, 0.5, None, ALU.mult)
                magict = wk.tile([128, cols], U32, tag=f"nm{cols}")
                nc.vector.memset(magict[:], MAGIC)
                y = wk.tile([128, cols], F32, tag=f"ny{cols}")
                yu = y[:].bitcast(U32)
                nc.vector.tensor_scalar(
                    yu, src.bitcast(U32), 1, None, ALU.logical_shift_right
                )
                nc.vector.tensor_tensor(yu, magict[:], yu, ALU.subtract)
                t1 = wk.tile([128, cols], F32, tag=f"nt{cols}")
                for _ in range(2):
                    nc.vector.tensor_tensor(t1[:], y[:], y[:], ALU.mult)
                    nc.vector.tensor_tensor(t1[:], half[:], t1[:], ALU.mult)
                    nc.vector.tensor_scalar(
                        t1[:], t1[:], -1.0, 1.5, ALU.mult, ALU.add
                    )
                    nc.vector.tensor_tensor(y[:], y[:], t1[:], ALU.mult)
                nc.vector.tensor_copy(dst, y[:])

            rnorm = sg.tile([128, NU], F32)
            rsqrt(rnorm[:], n2[:], NU)
            rnormo = sg.tile([128, NUO], F32)
            rsqrt(rnormo[:], n2o[:], NUO)

            # rnorm-scaled one-hots for the cluster-mean sums
            ohs = sg.tile([128, NU, K], BF16)
            nc.vector.tensor_tensor(
                ohs[:], oht[:], _expand(rnorm[:], K, 2), ALU.mult
            )

            # cluster counts: 4 colsum matmuls, (k,u) order per batch strip
            pcnt = psB.tile([128, 512], F32, tag="ce")
            for q in range(B):
                nc.tensor.matmul(
                    pcnt[32 * q : 32 * q + 1, :].rearrange(
                        "p (k u) -> p k u", k=K
                    ),
                    t_ones[:],
                    oht[:, 16 * q : 16 * (q + 1), :].rearrange("p u k -> p k u"),
                    start=True, stop=True,
                    tile_position=(0, 32 * q),
                )
            cs128 = sg.tile([128, 512], F32)
            nc.vector.tensor_copy(cs128[:], pcnt[:])
            cbk = sg.tile([128, 16], F32)
            for q in range(B):
                nc.gpsimd.dma_start(
                    out=cbk[32 * q : 32 * (q + 1), :],
                    in_=cs128[32 * q : 32 * q + 1, :].rearrange(
                        "b (k u) -> b k u", u=16
                    ),
                )
            ccnt = sg.tile([128, 1], F32)
            nc.vector.tensor_reduce(ccnt[:], cbk[:], mybir.AxisListType.X, ALU.add)
            ccl = sg.tile([128, 1], F32)
            nc.vector.tensor_scalar(ccl[:], ccnt[:], 1.0, None, ALU.max)
            recip = sg.tile([128, 1], F32)
            nc.vector.reciprocal(recip[:], ccl[:])

            # cluster-mean sums: 64 chunk matmuls into per-batch strips
            pmean = psM.tile([128, C], F32, tag="means")
            for u in range(NU):
                b = u // 16
                nc.tensor.matmul(
                    pmean[32 * b : 32 * (b + 1), :],
                    ohs[:, u, :],
                    ceft[:, u, :],
                    start=(u % 16 == 0), stop=(u % 16 == 15),
                    tile_position=(0, 32 * b),
                    skip_group_check=True,
                )
            msb = sg.tile([128, C], BF16)
            nc.vector.tensor_scalar(msb[:], pmean[:], recip[:], None, ALU.mult)

            # cT chunks for own rows with rnorm folded via diag transposes
            ct = sg.tile([C, ROWS], BF16)
            for g in range(2):
                ctps = psB.tile([C, 512], F32, tag="ce")
                for j in range(4):
                    u = g * 4 + j
                    diag = wk.tile([128, 128], BF16, tag="diag")
                    nc.vector.tensor_scalar(
                        diag[:], t_id1[:], rnormo[:, u : u + 1], None, ALU.mult
                    )
                    nc.tensor.matmul(
                        ctps[:, j * 128 : (j + 1) * 128],
                        cefot[:, u, :],
                        diag[:],
                        start=True, stop=True,
                    )
                nc.vector.tensor_copy(ct[:, g * 512 : (g + 1) * 512], ctps[:])

            # meansC = msb^T : [C, B*K]
            pmc = psB.tile([C, 128], F32, tag="ce")
            nc.tensor.matmul(pmc[:], msb[:], t_id1[:], start=True, stop=True)
            meansC = sg.tile([C, 128], BF16)
            nc.vector.tensor_copy(meansC[:], pmc[:])

            # logits for own rows: z[p=row, f=bk] in one [128, 1024] psum
            lg = psL.tile([128, 8 * 128], F32, tag="lg")
            for j in range(NUO):
                nc.tensor.matmul(
                    lg[:, j * 128 : (j + 1) * 128],
                    ct[:, j * 128 : (j + 1) * 128],
                    meansC[:],
                    start=True, stop=True,
                )
            ez = sg.tile([128, 8 * 128], BF16)
            nc.scalar.activation(ez[:], lg[:], ACT.Exp)
            zs = sg.tile([128, NUO], F32)
            nc.vector.tensor_reduce(
                zs[:], ez[:].rearrange("p (u k) -> p u k", u=NUO),
                mybir.AxisListType.X, ALU.add,
            )
            lnsum = sg.tile([128, NUO], F32)
            nc.scalar.activation(lnsum[:], zs[:], ACT.Ln)

            # target logits: z[:, u, 0:K] * own-row onehot, reduced
            jt = sg.tile([128, NUO, K], F32)
            nc.vector.tensor_tensor(
                jt[:],
                lg[:].rearrange("p (u k) -> p u k", u=NUO)[:, :, 0:K],
                ohot[:],
                ALU.mult,
            )
            ztgt = sg.tile([128, NUO], F32)
            nc.vector.tensor_reduce(ztgt[:], jt[:], mybir.AxisListType.X, ALU.add)

            # mask count: merge 4 accumulators, one cache-reduce
            mm0 = sg.tile([128, N], BF16)
            nc.vector.tensor_tensor(mm0[:], mc[0][:], mc[1][:], ALU.add)
            mm1 = sg.tile([128, N], BF16)
            nc.vector.tensor_tensor(mm1[:], mc[2][:], mc[3][:], ALU.add)
            mcs = sg.tile([128, N], BF16)
            nc.vector.tensor_tensor(mcs[:], mm0[:], mm1[:], ALU.add)
            junkc = sg.tile([128, N], BF16)
            cnt = sg.tile([128, 1], F32)
            nc.vector.tensor_scalar(
                junkc[:], mcs[:], 1.0, 0.0, ALU.mult, ALU.add, accum_out=cnt[:]
            )

            # ---------------- outputs ----------------------------------
            nc.gpsimd.dma_start(out=out_ext.ap()[:, 0:16], in_=emsum[:])
            nc.gpsimd.dma_start(out=out_ext.ap()[:, 16:24], in_=lnsum[:])
            nc.gpsimd.dma_start(out=out_ext.ap()[:, 24:32], in_=ztgt[:])
            nc.gpsimd.dma_start(out=out_ext.ap()[:, 32:33], in_=cnt[:])

    nc.compile()
    return nc


def _make_in_maps(embedding, contr_emb, abs_coords, patch_mask, cluster_labels):
    embedding = np.asarray(embedding, dtype=np.float32)
    contr_emb = np.asarray(contr_emb, dtype=np.float32)
    abs_coords = np.asarray(abs_coords, dtype=np.float32)
    patch_mask = np.asarray(patch_mask, dtype=np.int32)
    cluster_labels = np.asarray(cluster_labels, dtype=np.int32)

    f8 = mybir.dt.np(FP8)
    b16 = mybir.dt.np(BF16)

    ce_all = contr_emb.reshape(B * N, C)
    cef_h = np.ascontiguousarray(
        ce_all.reshape(NU, 128, C).transpose(1, 0, 2).reshape(128, NU * C)
    ).astype(b16)
    lab_all = cluster_labels.reshape(B * N)
    oh_full = (lab_all[:, None] == np.arange(K)[None, :]).astype(np.float32)
    oh_h = np.ascontiguousarray(
        oh_full.reshape(NU, 128, K).transpose(1, 0, 2).reshape(128, NU * K)
    ).astype(b16)
    cef_chunks = ce_all.reshape(NU, 128, C).transpose(1, 0, 2)  # [p, u, C]
    oh_chunks = oh_full.reshape(NU, 128, K).transpose(1, 0, 2)  # [p, u, K]

    identf8 = (np.eye(128, dtype=np.float32) * VMASK).astype(f8)
    ident1 = np.eye(128, dtype=np.float32).astype(b16)
    onesb = np.ones((128, 1), np.float32).astype(b16)
    negones = (-np.ones((2, N), np.float32)).astype(b16)

    in_maps = []
    for c in range(NC):
        b, h = c // 2, c % 2
        r0 = h * ROWS
        mf8 = patch_mask[b, r0 : r0 + ROWS, :].astype(np.float32).astype(f8)
        in_maps.append(
            {
                "maskf": mf8,
                "masku": mf8,
                "embjT": np.ascontiguousarray(embedding[b].T).astype(b16),
                "crdjT": np.ascontiguousarray(abs_coords[b].T).astype(b16),
                "embiT": np.ascontiguousarray(
                    embedding[b, r0 : r0 + ROWS].T
                ).astype(b16),
                "crdiT": np.ascontiguousarray(
                    abs_coords[b, r0 : r0 + ROWS].T
                ).astype(b16),
                "cef": cef_h,
                "oh": oh_h,
                "cefo": np.ascontiguousarray(
                    cef_chunks[:, c * NUO : (c + 1) * NUO, :].reshape(128, NUO * C)
                ).astype(b16),
                "oho": np.ascontiguousarray(
                    oh_chunks[:, c * NUO : (c + 1) * NUO, :].reshape(128, NUO * K)
                ).astype(b16),
                "identf8": identf8,
                "ident1": ident1,
                "onesb": onesb,
                "negones": negones,
            }
        )
    return in_maps


def _combine(results):
    s_em = 0.0
    s_cnt = 0.0
    s_ln = 0.0
    s_zt = 0.0
    for r in results:
        o = np.asarray(r["out"], dtype=np.float64)
        s_em += o[:, 0:16].sum()
        s_ln += o[:, 16:24].sum()
        s_zt += o[:, 24:32].sum()
        s_cnt += o[:, 32].sum()
    anchor = (s_cnt - s_em) / s_cnt
    bce = (s_ln - s_zt) / (B * N)
    return np.float32(anchor + CE_W * bce)


def run(inputs, trace=False, trace_kwargs=None):
    global _cached_nc
    if _cached_nc is None:
        _cached_nc = build()
    in_maps = _make_in_maps(**inputs)
    res = run_bass_kernel_spmd(
        _cached_nc, in_maps, list(range(NC)), trace=trace, **(trace_kwargs or {})
    )
    return _combine(res.results), res


def kernel(embedding, contr_emb, abs_coords, patch_mask, cluster_labels):
    out, _ = run(
        dict(
            embedding=embedding,
            contr_emb=contr_emb,
            abs_coords=abs_coords,
            patch_mask=patch_mask,
            cluster_labels=cluster_labels,
        )
    )
    return out


# revision 30
# speedup vs baseline: 1.1219x; 1.1219x over previous
"""Trainium2 Bass kernel for AnchorPlusContrastiveLoss (8 NeuronCores).

v3 design — no collective, everything replicated:
  Anchor: core c handles batch b=c//2, i-rows [h*1024,(h+1)*1024), h=c%2.
    P = 2x_i.x_j - r_i - r_j via one K=6 matmul
    (lhsT=[2x0,2x1,sq0,sq1,-1,-1], rhs=[x0,x1,-1,-1,sq0,sq1]) plus V*m via
    a K=128 fp8 matmul (diag=240, mask {0,1} fp8).  ScalarE
    exp(P/10 - 24) with accum_out yields per-partition sums of e*m
    (masked-out lanes underflow to exactly 0).
  Mask count: the fp8 mask is fetched a second time through four SWDGE
    cast+accumulate DMA chains into bf16 accumulators; a small DVE merge
    tree + one cache-reduce finishes the count.
  CE: every core computes all-batch cluster means locally from the full
    contr_emb (bf16) + host one-hot labels; per-batch strips of one
    [128,64] PSUM tile accumulate rnorm-weighted cluster sums; cluster
    counts via 4 tiny colsum matmuls; rnorm via DVE Newton rsqrt (no
    Ln/Exp table ping-pong); logits for own 1024 rows in [row, b*K]
    orientation; 1/count folded into meansC, rnorm folded into PE diag
    transposes of cT.
  Host combine: sums tiny per-core partials (scalars only).
"""

import numpy as np

import concourse.bacc as bacc
import concourse.bass as bass
import concourse.tile as tile
from concourse import mybir
from concourse.bass_utils import run_bass_kernel_spmd

F32 = mybir.dt.float32
BF16 = mybir.dt.bfloat16
FP8 = mybir.dt.float8e4
U32 = mybir.dt.uint32
I32 = mybir.dt.int32
ALU = mybir.AluOpType
ACT = mybir.ActivationFunctionType

B, N, D, C, K = 4, 2048, 2, 64, 32
NC = 8
ROWS = N // 2          # 1024 anchor i-rows per core
NT = ROWS // 128       # 8 i-tiles per core
NU = (B * N) // 128    # 64 row-chunks of the full contr_emb
NUO = ROWS // 128      # 8 own row-chunks for the CE tail
VMASK = 240.0          # fp8-exact mask amplitude; exp(-24) ~ 3.8e-11
TEMP = 10.0
CE_W = 10.0
MAGIC = 0x5F3759DF

_cached_nc = None


def _expand(ap, reps, where):
    """Insert a 0-stride dim of length `reps` at position `where`."""
    aps = list(ap.ap)
    aps.insert(where, [0, reps])
    return bass.AP(tensor=ap.tensor, offset=ap.offset, ap=aps)


def build():
    nc = bacc.Bacc("TRN2", target_bir_lowering=False, debug=False, num_devices=NC)

    maskf = nc.declare_dram_parameter("maskf", [ROWS, N], FP8, isOutput=False)
    masku = nc.declare_dram_parameter("masku", [ROWS, N], FP8, isOutput=False)
    embjT = nc.declare_dram_parameter("embjT", [D, N], BF16, isOutput=False)
    crdjT = nc.declare_dram_parameter("crdjT", [D, N], BF16, isOutput=False)
    embiT = nc.declare_dram_parameter("embiT", [D, ROWS], BF16, isOutput=False)
    crdiT = nc.declare_dram_parameter("crdiT", [D, ROWS], BF16, isOutput=False)
    cef = nc.declare_dram_parameter("cef", [128, NU * C], BF16, isOutput=False)
    oh = nc.declare_dram_parameter("oh", [128, NU * K], BF16, isOutput=False)
    cefo = nc.declare_dram_parameter("cefo", [128, NUO * C], BF16, isOutput=False)
    oho = nc.declare_dram_parameter("oho", [128, NUO * K], BF16, isOutput=False)
    identf8 = nc.declare_dram_parameter("identf8", [128, 128], FP8, isOutput=False)
    ident1 = nc.declare_dram_parameter("ident1", [128, 128], BF16, isOutput=False)
    onesb = nc.declare_dram_parameter("onesb", [128, 1], BF16, isOutput=False)
    negones = nc.declare_dram_parameter("negones", [2, N], BF16, isOutput=False)
    out_ext = nc.declare_dram_parameter("out", [128, 34], F32, isOutput=True)

    with tile.TileContext(nc) as tc:
        with (
            tc.tile_pool(name="singles", bufs=1) as sg,
            tc.tile_pool(name="work", bufs=1) as wk,
            tc.tile_pool(name="maskp", bufs=3) as mp,
            tc.tile_pool(name="ep", bufs=2) as ep,
            tc.tile_pool(name="psA", bufs=2, space="PSUM") as psA,
            tc.tile_pool(name="psB", bufs=1, space="PSUM") as psB,
            tc.tile_pool(name="psM", bufs=1, space="PSUM") as psM,
            tc.tile_pool(name="psL", bufs=1, space="PSUM") as psL,
        ):
            # ---- ScalarE exp-table preload via dummy act ----
            dume = sg.tile([1, 1], F32)
            nc.vector.memset(dume[:], 1.0)
            dumo = sg.tile([1, 1], F32)
            nc.scalar.activation(dumo[:], dume[:], ACT.Exp)

            # ---------------- small input DMAs (sync queue) ----------
            lhsT6 = sg.tile([6, ROWS], BF16)
            rhs6 = sg.tile([6, N], BF16)
            nc.sync.dma_start(out=lhsT6[4:6, :], in_=negones.ap()[:, 0:ROWS])
            nc.sync.dma_start(out=rhs6[2:4, :], in_=negones.ap())
            etj = wk.tile([2, N], BF16)
            nc.sync.dma_start(out=etj[:], in_=embjT.ap())
            atj = wk.tile([2, N], BF16)
            nc.sync.dma_start(out=atj[:], in_=crdjT.ap())
            eti = wk.tile([2, ROWS], BF16)
            nc.sync.dma_start(out=eti[:], in_=embiT.ap())
            ati = wk.tile([2, ROWS], BF16)
            nc.sync.dma_start(out=ati[:], in_=crdiT.ap())
            t_if8 = sg.tile([128, 128], FP8)
            nc.sync.dma_start(out=t_if8[:], in_=identf8.ap())
            t_id1 = sg.tile([128, 128], BF16)
            nc.sync.dma_start(out=t_id1[:], in_=ident1.ap())
            t_ones = sg.tile([128, 1], BF16)
            nc.sync.dma_start(out=t_ones[:], in_=onesb.ap())

            # ---------------- anchor operand prep (all DVE + 2 DMAs) ----
            xt = wk.tile([2, N], BF16)
            nc.vector.tensor_tensor(xt[:], etj[:], atj[:], ALU.add)
            nc.vector.tensor_copy(rhs6[0:2, :], xt[:])
            sqj = wk.tile([2, N], BF16)
            nc.vector.tensor_tensor(sqj[:], xt[:], xt[:], ALU.mult)
            nc.sync.dma_start(out=rhs6[4:6, :], in_=sqj[:])

            xo = wk.tile([2, ROWS], BF16)
            nc.vector.tensor_tensor(xo[:], eti[:], ati[:], ALU.add)
            nc.vector.tensor_scalar(lhsT6[0:2, :], xo[:], 2.0, None, ALU.mult)
            sqi = wk.tile([2, ROWS], BF16)
            nc.vector.tensor_tensor(sqi[:], xo[:], xo[:], ALU.mult)
            nc.sync.dma_start(out=lhsT6[2:4, :], in_=sqi[:])

            # ---------------- bulk input DMAs (sync queue) -------------
            mt = []
            for t in range(NT):
                m = mp.tile([128, N], FP8, tag="mask")
                nc.sync.dma_start(
                    out=m[:], in_=maskf.ap().rearrange("(t p) n -> t p n", p=128)[t]
                )
                mt.append(m)
                if t == 0:
                    ceft = sg.tile([128, NU, C], BF16)
                    nc.sync.dma_start(
                        out=ceft[:],
                        in_=cef.ap().rearrange("p (u c) -> p u c", u=NU),
                    )
                if t == 1:
                    oht = sg.tile([128, NU, K], BF16)
                    nc.sync.dma_start(
                        out=oht[:], in_=oh.ap().rearrange("p (u k) -> p u k", u=NU)
                    )
                    cefot = sg.tile([128, NUO, C], BF16)
                    nc.sync.dma_start(
                        out=cefot[:],
                        in_=cefo.ap().rearrange("p (u c) -> p u c", u=NUO),
                    )
                    ohot = sg.tile([128, NUO, K], BF16)
                    nc.sync.dma_start(
                        out=ohot[:],
                        in_=oho.ap().rearrange("p (u k) -> p u k", u=NUO),
                    )

            # ---- mask count: four cast+accumulate SWDGE chains --------
            mc = [
                sg.tile([128, N], BF16, name=f"mc{i}", tag=f"mc{i}")
                for i in range(4)
            ]
            mview = masku.ap().rearrange("(t p) n -> t p n", p=128)
            for t in range(NT):
                nc.gpsimd.dma_start(
                    out=mc[t % 4][:],
                    in_=mview[t],
                    accum_op=(ALU.bypass if t < 4 else ALU.add),
                )

            # ---------------- anchor main loop --------------------------
            emsum = sg.tile([128, 2 * NT], F32)
            bias24 = sg.tile([128, 1], F32)
            nc.vector.memset(bias24[:], -VMASK / TEMP)
            for t in range(NT):
                for h in range(2):
                    pt = psA.tile([128, 1024], F32, tag="anchor")
                    for q in range(2):
                        cs = slice((2 * h + q) * 512, (2 * h + q + 1) * 512)
                        nc.tensor.matmul(
                            pt[:, q * 512 : (q + 1) * 512],
                            lhsT6[:, t * 128 : (t + 1) * 128],
                            rhs6[:, cs],
                            start=True, stop=False,
                            skip_group_check=True,
                        )
                    for q in range(2):
                        cs = slice((2 * h + q) * 512, (2 * h + q + 1) * 512)
                        nc.tensor.matmul(
                            pt[:, q * 512 : (q + 1) * 512],
                            t_if8[:],
                            mt[t][:, cs],
                            start=False, stop=True,
                            skip_group_check=True,
                        )
                    je = ep.tile([128, 1024], BF16, tag="junk")
                    nc.scalar.activation(
                        je[:], pt[:], ACT.Exp,
                        bias=bias24[:],
                        scale=1.0 / TEMP,
                        accum_out=emsum[:, 2 * t + h : 2 * t + h + 1],
                    )

            # ---------------- CE head -----------------------------------
            # squared norms for all B*N rows: bf16 square + tree reduce
            sq = sg.tile([128, NU, C], BF16)
            nc.vector.tensor_tensor(sq[:], ceft[:], ceft[:], ALU.mult)
            s32 = sg.tile([128, NU, 32], BF16)
            nc.vector.tensor_tensor(
                s32[:],
                sq[:].rearrange("p u (a c) -> p u a c", a=2)[:, :, 0],
                sq[:].rearrange("p u (a c) -> p u a c", a=2)[:, :, 1],
                ALU.add,
            )
            s16 = sg.tile([128, NU, 16], BF16)
            nc.vector.tensor_tensor(
                s16[:],
                s32[:].rearrange("p u (a c) -> p u a c", a=2)[:, :, 0],
                s32[:].rearrange("p u (a c) -> p u a c", a=2)[:, :, 1],
                ALU.add,
            )
            n2 = sg.tile([128, NU], F32)
            nc.vector.tensor_reduce(n2[:], s16[:], mybir.AxisListType.X, ALU.add)
            # own-row norms (separate small input)
            sqo = sg.tile([128, NUO, C], BF16)
            nc.vector.tensor_tensor(sqo[:], cefot[:], cefot[:], ALU.mult)
            so32 = sg.tile([128, NUO, 32], BF16)
            nc.vector.tensor_tensor(
                so32[:],
                sqo[:].rearrange("p u (a c) -> p u a c", a=2)[:, :, 0],
                sqo[:].rearrange("p u (a c) -> p u a c", a=2)[:, :, 1],
                ALU.add,
            )
            n2o = sg.tile([128, NUO], F32)
            nc.vector.tensor_reduce(n2o[:], so32[:], mybir.AxisListType.X, ALU.add)

            # rnorm = rsqrt(n2) via magic-constant Newton (2 iterations)
            def rsqrt(dst, src, cols):
                half = wk.tile([128, cols], F32, tag=f"nh{cols}")
                nc.vector.tensor_scalar(half[:], src, 0.5, None, ALU.mult)
                magict = wk.tile([128, cols], U32, tag=f"nm{cols}")
                nc.vector.memset(magict[:], MAGIC)
                y = wk.tile([128, cols], F32, tag=f"ny{cols}")
                yu = y[:].bitcast(U32)
                nc.vector.tensor_scalar(
                    yu, src.bitcast(U32), 1, None, ALU.logical_shift_right
                )
                nc.vector.tensor_tensor(yu, magict[:], yu, ALU.subtract)
                t1 = wk.tile([128, cols], F32, tag=f"nt{cols}")
                for _ in range(2):
                    nc.vector.tensor_tensor(t1[:], y[:], y[:], ALU.mult)
                    nc.vector.tensor_tensor(t1[:], half[:], t1[:], ALU.mult)
                    nc.vector.tensor_scalar(
                        t1[:], t1[:], -1.0, 1.5, ALU.mult, ALU.add
                    )
                    nc.vector.tensor_tensor(y[:], y[:], t1[:], ALU.mult)
                nc.vector.tensor_copy(dst, y[:])

            rnorm = sg.tile([128, NU], F32)
            rsqrt(rnorm[:], n2[:], NU)
            rnormo = sg.tile([128, NUO], F32)
            rsqrt(rnormo[:], n2o[:], NUO)

            # rnorm-scaled one-hots for the cluster-mean sums
            ohs = sg.tile([128, NU, K], BF16)
            nc.vector.tensor_tensor(
                ohs[:], oht[:], _expand(rnorm[:], K, 2), ALU.mult
            )

            # cluster counts: 4 colsum matmuls, (k,u) order per batch strip
            pcnt = psB.tile([128, 512], F32, tag="ce")
            for q in range(B):
                nc.tensor.matmul(
                    pcnt[32 * q : 32 * q + 1, :].rearrange(
                        "p (k u) -> p k u", k=K
                    ),
                    t_ones[:],
                    oht[:, 16 * q : 16 * (q + 1), :].rearrange("p u k -> p k u"),
                    start=True, stop=True,
                    tile_position=(0, 32 * q),
                )
            cs128 = sg.tile([128, 512], F32)
            nc.vector.tensor_copy(cs128[:], pcnt[:])
            cbk = sg.tile([128, 16], F32)
            for q in range(B):
                nc.gpsimd.dma_start(
                    out=cbk[32 * q : 32 * (q + 1), :],
                    in_=cs128[32 * q : 32 * q + 1, :].rearrange(
                        "b (k u) -> b k u", u=16
                    ),
                )
            ccnt = sg.tile([128, 1], F32)
            nc.vector.tensor_reduce(ccnt[:], cbk[:], mybir.AxisListType.X, ALU.add)
            ccl = sg.tile([128, 1], F32)
            nc.vector.tensor_scalar(ccl[:], ccnt[:], 1.0, None, ALU.max)
            recip = sg.tile([128, 1], F32)
            nc.vector.reciprocal(recip[:], ccl[:])

            # cluster-mean sums: 64 chunk matmuls into per-batch strips
            pmean = psM.tile([128, C], F32, tag="means")
            for u in range(NU):
                b = u // 16
                nc.tensor.matmul(
                    pmean[32 * b : 32 * (b + 1), :],
                    ohs[:, u, :],
                    ceft[:, u, :],
                    start=(u % 16 == 0), stop=(u % 16 == 15),
                    tile_position=(0, 32 * b),
                    skip_group_check=True,
                )
            msb = sg.tile([128, C], BF16)
            nc.vector.tensor_scalar(msb[:], pmean[:], recip[:], None, ALU.mult)

            # cT chunks for own rows with rnorm folded via diag transposes
            ct = sg.tile([C, ROWS], BF16)
            for g in range(2):
                ctps = psB.tile([C, 512], F32, tag="ce")
                for j in range(4):
                    u = g * 4 + j
                    diag = wk.tile([128, 128], BF16, tag="diag")
                    nc.vector.tensor_scalar(
                        diag[:], t_id1[:], rnormo[:, u : u + 1], None, ALU.mult
                    )
                    nc.tensor.matmul(
                        ctps[:, j * 128 : (j + 1) * 128],
                        cefot[:, u, :],
                        diag[:],
                        start=True, stop=True,
                    )
                nc.vector.tensor_copy(ct[:, g * 512 : (g + 1) * 512], ctps[:])

            # meansC = msb^T : [C, B*K]
            pmc = psB.tile([C, 128], F32, tag="ce")
            nc.tensor.matmul(pmc[:], msb[:], t_id1[:], start=True, stop=True)
            meansC = sg.tile([C, 128], BF16)
            nc.vector.tensor_copy(meansC[:], pmc[:])

            # logits for own rows: z[p=row, f=bk] in one [128, 1024] psum
            lg = psL.tile([128, 8 * 128], F32, tag="lg")
            for j in range(NUO):
                nc.tensor.matmul(
                    lg[:, j * 128 : (j + 1) * 128],
                    ct[:, j * 128 : (j + 1) * 128],
                    meansC[:],
                    start=True, stop=True,
                )
            ez = sg.tile([128, 8 * 128], BF16)
            nc.scalar.activation(ez[:], lg[:], ACT.Exp)
            zs = sg.tile([128, NUO], F32)
            nc.vector.tensor_reduce(
                zs[:], ez[:].rearrange("p (u k) -> p u k", u=NUO),
                mybir.AxisListType.X, ALU.add,
            )
            lnsum = sg.tile([128, NUO], F32)
            nc.scalar.activation(lnsum[:], zs[:], ACT.Ln)

            # target logits: z[:, u, 0:K] * own-row onehot, reduced
            jt = sg.tile([128, NUO, K], F32)
            nc.vector.tensor_tensor(
                jt[:],
                lg[:].rearrange("p (u k) -> p u k", u=NUO)[:, :, 0:K],
                ohot[:],
                ALU.mult,
            )
            ztgt = sg.tile([128, NUO], F32)
            nc.vector.tensor_reduce(ztgt[:], jt[:], mybir.AxisListType.X, ALU.add)

            # mask count: merge 4 accumulators, one cache-reduce
            mm0 = sg.tile([128, N], BF16)
            nc.vector.tensor_tensor(mm0[:], mc[0][:], mc[1][:], ALU.add)
            mm1 = sg.tile([128, N], BF16)
            nc.vector.tensor_tensor(mm1[:], mc[2][:], mc[3][:], ALU.add)
            mcs = sg.tile([128, N], BF16)
            nc.vector.tensor_tensor(mcs[:], mm0[:], mm1[:], ALU.add)
            junkc = sg.tile([128, N], BF16)
            cnt = sg.tile([128, 1], F32)
            nc.vector.tensor_scalar(
                junkc[:], mcs[:], 1.0, 0.0, ALU.mult, ALU.add, accum_out=cnt[:]
            )

            # ---------------- outputs ----------------------------------
            nc.gpsimd.dma_start(out=out_ext.ap()[:, 0:16], in_=emsum[:])
            nc.gpsimd.dma_start(out=out_ext.ap()[:, 16:24], in_=lnsum[:])
            nc.gpsimd.dma_start(out=out_ext.ap()[:, 24:32], in_=ztgt[:])
            nc.gpsimd.dma_start(out=out_ext.ap()[:, 32:33], in_=cnt[:])

    nc.compile()
    return nc


def _make_in_maps(embedding, contr_emb, abs_coords, patch_mask, cluster_labels):
    embedding = np.asarray(embedding, dtype=np.float32)
    contr_emb = np.asarray(contr_emb, dtype=np.float32)
    abs_coords = np.asarray(abs_coords, dtype=np.float32)
    patch_mask = np.asarray(patch_mask, dtype=np.int32)
    cluster_labels = np.asarray(cluster_labels, dtype=np.int32)

    f8 = mybir.dt.np(FP8)
    b16 = mybir.dt.np(BF16)

    ce_all = contr_emb.reshape(B * N, C)
    cef_h = np.ascontiguousarray(
        ce_all.reshape(NU, 128, C).transpose(1, 0, 2).reshape(128, NU * C)
    ).astype(b16)
    lab_all = cluster_labels.reshape(B * N)
    oh_full = (lab_all[:, None] == np.arange(K)[None, :]).astype(np.float32)
    oh_h = np.ascontiguousarray(
        oh_full.reshape(NU, 128, K).transpose(1, 0, 2).reshape(128, NU * K)
    ).astype(b16)
    cef_chunks = ce_all.reshape(NU, 128, C).transpose(1, 0, 2)  # [p, u, C]
    oh_chunks = oh_full.reshape(NU, 128, K).transpose(1, 0, 2)  # [p, u, K]

    identf8 = (np.eye(128, dtype=np.float32) * VMASK).astype(f8)
    ident1 = np.eye(128, dtype=np.float32).astype(b16)
    onesb = np.ones((128, 1), np.float32).astype(b16)
    negones = (-np.ones((2, N), np.float32)).astype(b16)

    in_maps = []
    for c in range(NC):
        b, h = c // 2, c % 2
        r0 = h * ROWS
        mf8 = patch_mask[b, r0 : r0 + ROWS, :].astype(np.float32).astype(f8)
        in_maps.append(
            {
                "maskf": mf8,
                "masku": mf8,
                "embjT": np.ascontiguousarray(embedding[b].T).astype(b16),
                "crdjT": np.ascontiguousarray(abs_coords[b].T).astype(b16),
                "embiT": np.ascontiguousarray(
                    embedding[b, r0 : r0 + ROWS].T
                ).astype(b16),
                "crdiT": np.ascontiguousarray(
                    abs_coords[b, r0 : r0 + ROWS].T
                ).astype(b16),
                "cef": cef_h,
                "oh": oh_h,
                "cefo": np.ascontiguousarray(
                    cef_chunks[:, c * NUO : (c + 1) * NUO, :].reshape(128, NUO * C)
                ).astype(b16),
                "oho": np.ascontiguousarray(
                    oh_chunks[:, c * NUO : (c + 1) * NUO, :].reshape(128, NUO * K)
                ).astype(b16),
                "identf8": identf8,
                "ident1": ident1,
                "onesb": onesb,
                "negones": negones,
            }
        )
    return in_maps


def _combine(results):
    s_em = 0.0
    s_cnt = 0.0
    s_ln = 0.0
    s_zt = 0.0
    for r in results:
        o = np.asarray(r["out"], dtype=np.float64)
        s_em += o[:, 0:16].sum()
        s_ln += o[:, 16:24].sum()
        s_zt += o[:, 24:32].sum()
        s_cnt += o[:, 32].sum()
    anchor = (s_cnt - s_em) / s_cnt
    bce = (s_ln - s_zt) / (B * N)
    return np.float32(anchor + CE_W * bce)


def run(inputs, trace=False, trace_kwargs=None):
    global _cached_nc
    if _cached_nc is None:
        _cached_nc = build()
    in_maps = _make_in_maps(**inputs)
    res = run_bass_kernel_spmd(
        _cached_nc, in_maps, list(range(NC)), trace=trace, **(trace_kwargs or {})
    )
    return _combine(res.results), res


def kernel(embedding, contr_emb, abs_coords, patch_mask, cluster_labels):
    out, _ = run(
        dict(
            embedding=embedding,
            contr_emb=contr_emb,
            abs_coords=abs_coords,
            patch_mask=patch_mask,
            cluster_labels=cluster_labels,
        )
    )
    return out
